# revision 46
# baseline (speedup 1.0000x reference)
"""Trainium2 Bass kernel for nn_BitwiseLinear (8 NeuronCores, SPMD).

Reference semantics (B=32768, IN=OUT=1024):
    out = in_scale * weight_scale * (sign(x) @ sign(weight * gate_mask).T + bias)
    gate_mask = (sign(gate)+1)/2; in_scale = mean|x| per row; weight_scale = mean|w| per out.

Identities used:
    sign(weight * gate_mask) == sign(weight) * (gate >= 0)  (gate==0 -> mask 0.5 -> sign(w))
    out = sum|x|_row * (signmm + bias) * ws_eff,  ws_eff = sum|w|_row * 2^-20

Sharding: data-parallel on batch across the 8 cores (x/out sharded 4096 rows
per core), weight replicated, no collectives.

Design (vs the f32 PE-transpose baseline):
  * x and weight are cast to bf16 on the host (2e-2 tolerance; bf16 costs
    ~0.25%): halves the x/w HBM reads; output written bf16, upcast on host.
  * All x/w transposes moved off the PE onto the DMA XBAR transpose
    (2-byte dtype): weight is transposed [o,i]->[i,o] straight from DRAM
    in two k-half descriptors on the scalar hwdge queue; x groups go
    DRAM->SBUF plain on the scalar queue (the [b,i] layout feeds the
    per-row |x| sums on DVE) and SBUF->SBUF through the XBAR on the sync
    queue. (DRAM-sourced XBAR of the x shards corrupted data on
    even-numbered cores, so x stays SBUF-sourced.)
  * PE runs only the 512 fp8 DoubleRow matmuls (sign(x)^T stationary,
    [P, 2*NCH] psum tiles spanning two banks, 4 in flight).
  * Epilogue split for engine balance: most tiles fold the per-row |x|
    sum into the PSUM->bf16 move on ACT (per-partition scale AP) then
    multiply by ws in bf16 on Pool/DVE; the rest do psum*ws on DVE (f32)
    plus an in-place bf16 tensor_scalar. Row-sum reduces are emitted one
    per tile, two groups ahead, so the DVE queue never delays the
    psum-draining ops.
  Runtime specialization: bias matmuls dropped when bias is all-zero and the
  gate path dropped when gate >= 0 everywhere (checked against the actual
  inputs; other variants compile lazily and remain correct).
"""

import numpy as np
import ml_dtypes

import concourse.bacc as bacc
import concourse.mybir as mybir
import concourse.tile as tile
from concourse import masks
from concourse.bass_utils import run_bass_kernel_spmd

B, IN, OUT = 32768, 1024, 1024
NCORES = 8
BSH = B // NCORES            # 4096 rows per core
P = 128                      # partitions
NT = BSH // P                # 32 x-tiles per core
G = 4                        # x-tiles per DMA/XBAR group
NG = NT // G                 # 8 groups
KC = IN // P                 # 8 contraction chunks of 128
NPAIR = KC // 2              # 4 DoubleRow K-pairs (256 each)
NCH = 512                    # matmul moving free-dim (one PSUM bank of f32)
F32 = mybir.dt.float32
BF16 = mybir.dt.bfloat16
FP8 = mybir.dt.float8e4
WS_SCALE = float(2.0 ** -20)  # 1/(1024*1024): folds both mean divisors

_CACHE: dict = {}


def _build(with_bias=True, with_gate=True):
    nc = bacc.Bacc("TRN2", target_bir_lowering=False, debug=False,
                   num_devices=NCORES)

    x_ext = nc.declare_dram_parameter("x", [BSH, IN], BF16, isOutput=False)
    w_ext = nc.declare_dram_parameter("weight", [OUT, IN], BF16, isOutput=False)
    g_ext = nc.declare_dram_parameter("gate", [OUT, IN], BF16, isOutput=False)
    b_ext = nc.declare_dram_parameter("bias", [1, OUT], F32, isOutput=False)
    o_ext = nc.declare_dram_parameter("out", [BSH, OUT], BF16, isOutput=True)

    x_ap = x_ext.ap()
    w_ap = w_ext.ap()
    g_ap = g_ext.ap()
    b_ap = b_ext.ap()
    o_ap = o_ext.ap()

    ACT = mybir.ActivationFunctionType
    ALU = mybir.AluOpType
    AX = mybir.AxisListType
    DR = mybir.MatmulPerfMode.DoubleRow

    with tile.TileContext(nc) as tc:
        with tc.tile_pool(name="const", bufs=1) as cp:
            ident_f32 = cp.tile([P, P], F32)
            ones_f32 = cp.tile([1, P], F32)
            zbias = cp.tile([P, 1], F32)

            # persistent prepped weights: wT chunks c at [:, c*OUT:(c+1)*OUT]
            # (chunk c holds w[o, c*128+p] for all o)
            wT_bf = cp.tile([P, KC * OUT], BF16, name="wT_bf")
            wtq = cp.tile([P, KC * OUT], FP8, name="wtq")
            ws_bcast = cp.tile([P, OUT], F32)     # ws * 2^-20 broadcast
            ws_bcast_bf = cp.tile([P, OUT], BF16)
            if with_bias:
                ones_f8 = cp.tile([1, P], FP8)
                bias_f8 = cp.tile([1, OUT], FP8)

            wtq_v = wtq[:].rearrange("p (c o) -> p c o", c=KC)

            with tc.tile_pool(name="wk", bufs=1) as wk, \
                 tc.tile_pool(name="xin", bufs=4) as xin_pool, \
                 tc.tile_pool(name="xtr", bufs=4) as xtr_pool, \
                 tc.tile_pool(name="xbt", bufs=4) as xbt_pool, \
                 tc.tile_pool(name="pre", bufs=4) as pre_pool, \
                 tc.tile_pool(name="og", bufs=3) as og_pool, \
                 tc.tile_pool(name="sc", bufs=16) as sc_pool, \
                 tc.tile_pool(name="pso", bufs=4, space="PSUM") as pso_pool:

                xgs = [None] * NG
                xTs = [None] * NG
                sxTs = [None] * NG
                is_raws = [None] * NT

                def stage_in(g):
                    """x group g: plain DMA in (scalar hwdge), then SBUF->
                    SBUF XBAR transpose on the sync queue (DRAM-sourced
                    XBAR transposes corrupt data on even-numbered cores).
                    Resulting layout: [p, t*KC+c, b]."""
                    xg = xin_pool.tile([P, G * IN], BF16, tag="xg",
                                       name=f"xg{g}")
                    nc.scalar.dma_start(
                        xg[:].rearrange("p (t i) -> p t i", t=G),
                        x_ap[g * G * P:(g + 1) * G * P, :].rearrange(
                            "(t p) i -> p t i", p=P))
                    xT = xtr_pool.tile([P, KC * G * P], BF16, tag="xT",
                                       name=f"xT{g}")
                    nc.sync.dma_start(
                        out=xT[:].rearrange("p (c m) -> p c m", c=G * KC),
                        in_=xg[:],
                        transpose=True)
                    xgs[g] = xg
                    xTs[g] = xT

                def emit_reduce(it):
                    """per-row |x| sum for tile it; deferred into the main
                    loop so the DVE queue isn't clogged ahead of the
                    psum-draining epilogue ops."""
                    if it >= NT or is_raws[it] is not None:
                        return
                    g, t = divmod(it, G)
                    is_raw = sc_pool.tile([P, 1], F32, tag="is_raw",
                                          name=f"is_raw{it}")
                    nc.vector.tensor_reduce(
                        is_raw[:], xgs[g][:, t * IN:(t + 1) * IN],
                        axis=AX.X, op=ALU.add,
                        apply_absolute_value=True)
                    is_raws[it] = is_raw

                def sign_group(g):
                    sxT = xbt_pool.tile([P, KC * G * P], FP8, tag="sxT",
                                        name=f"sxT{g}")
                    nc.scalar.activation(sxT[:], xTs[g][:], ACT.Sign,
                                         bias=zbias[:])
                    sxTs[g] = sxT

                masks.make_identity(nc, ident_f32[:])
                nc.gpsimd.memset(ones_f32[:], 1.0)
                nc.gpsimd.memset(zbias[:], 0.0)
                if with_bias:
                    nc.gpsimd.memset(ones_f8[:], 1.0)

                # ---- prologue: x group 0 first on the scalar queue, then
                # the [o, i] -> [i, o] weight XBAR in two k-chunk halves so
                # Sign(w, half 0) lands as early as possible ----
                stage_in(0)
                wT_v = wT_bf[:].rearrange("p (c o) -> p c o", c=KC)
                for h in range(2):
                    nc.scalar.dma_start(
                        out=wT_v[:, h * 4:(h + 1) * 4, :],
                        in_=w_ap[:, h * IN // 2:(h + 1) * IN // 2],
                        transpose=True)
                stage_in(1)
                stage_in(2)
                # plain copy of w only to compute the per-o |w| row sums
                wt4 = [wk.tile([P, 4 * IN], BF16, tag=f"wt4_{q}",
                               name=f"wt4_{q}") for q in range(2)]
                for q in range(2):
                    nc.gpsimd.dma_start(
                        wt4[q][:].rearrange("p (t i) -> p t i", t=4),
                        w_ap[q * 4 * P:(q + 1) * 4 * P, :].rearrange(
                            "(t p) i -> p t i", p=P))
                ws_cols = wk.tile([P, KC], F32)   # per-o |w| row sums
                ws_row = wk.tile([1, OUT], F32)
                if with_bias:
                    bias_sb = wk.tile([1, OUT], F32)
                    nc.sync.dma_start(bias_sb[:], b_ap[:, :])
                    nc.vector.tensor_copy(bias_f8[:], bias_sb[:])
                if with_gate:
                    gT_bf = wk.tile([P, KC * OUT], BF16, name="gT_bf")
                    nc.scalar.dma_start(
                        out=gT_bf[:].rearrange("p (c o) -> p c o", c=KC),
                        in_=g_ap[:, :],
                        transpose=True)

                # weight sign binarization on ACT, k-chunk halves in j-order,
                # interleaved with the first x Signs on the in-order ACT
                # queue so matmuls(g0, j=0..1) start as early as possible.
                if with_gate:
                    msk = wk.tile([P, KC * OUT], BF16, name="gmask")
                    nc.vector.tensor_scalar(msk[:], gT_bf[:], 0.0, None,
                                            op0=ALU.is_ge)
                    wTm = wk.tile([P, KC * OUT], BF16, name="wTm")
                    nc.vector.tensor_tensor(wTm[:], wT_bf[:], msk[:],
                                            op=ALU.mult)
                    w_src = wTm
                else:
                    w_src = wT_bf
                half = KC * OUT // 2
                nc.scalar.activation(wtq[:, :half], w_src[:, :half], ACT.Sign,
                                     bias=zbias[:])
                sign_group(0)
                nc.scalar.activation(wtq[:, half:], w_src[:, half:], ACT.Sign,
                                     bias=zbias[:])
                sign_group(1)
                sign_group(2)
                # per-o |w| sums first on the DVE queue: they gate the ws
                # PE ops; the x row sums only feed epilogues
                for t in range(KC):
                    nc.vector.tensor_reduce(
                        ws_cols[:, t:t + 1],
                        wt4[t // 4][:, (t % 4) * IN:((t % 4) + 1) * IN],
                        axis=AX.X, op=ALU.add, apply_absolute_value=True)
                for it in range(2 * G):
                    emit_reduce(it)

                def emit_ws_prep():
                    """ws_row[0, o] = sum_i |w[o, i]| * 2^-20 via tiny PE
                    transposes + K=1 broadcast matmuls. Emitted mid-loop
                    (after 2 x-tiles of matmuls, before their epilogues) so
                    this chain never heads the in-order PE queue and stalls
                    it on the w-reduce dependencies. Exactly 2 psum ring
                    slots so tiles 0/1's psum isn't recycled early."""
                    ps_wt = pso_pool.tile([P, 2 * NCH], F32, tag="ps_o",
                                          name="ps_row")
                    for t in range(KC):
                        nc.tensor.transpose(ps_wt[0:1, t * P:(t + 1) * P],
                                            ws_cols[:, t:t + 1],
                                            ident_f32[:])
                    nc.scalar.activation(ws_row[:], ps_wt[0:1, :],
                                         ACT.Copy, scale=WS_SCALE)
                    ps_bc = pso_pool.tile([P, 2 * NCH], F32, tag="ps_o",
                                          name="ps_bc")
                    for n in range(OUT // NCH):
                        nc.tensor.matmul(ps_bc[:, n * NCH:(n + 1) * NCH],
                                         ones_f32[:],
                                         ws_row[:, n * NCH:(n + 1) * NCH])
                    nc.vector.tensor_copy(ws_bcast[:], ps_bc[:])
                    nc.vector.tensor_copy(ws_bcast_bf[:], ws_bcast[:])

                def emit_matmuls(g, t):
                    # one [P, 1024] f32 psum tile = two adjacent banks;
                    # each matmul writes within a single bank, epilogue
                    # ops span both in one instruction
                    sxv = sxTs[g][:].rearrange("p (c m) -> p c m", c=G * KC)
                    ps = pso_pool.tile([P, 2 * NCH], F32, tag="ps_o")
                    for j in range(NPAIR):
                        xp = sxv[:, t * KC + 2 * j:t * KC + 2 * j + 2, :]
                        wq = wtq_v[:, 2 * j:2 * j + 2, :]
                        for n in range(OUT // NCH):
                            nc.tensor.matmul(
                                ps[:, n * NCH:(n + 1) * NCH],
                                xp,
                                wq[:, :, n * NCH:(n + 1) * NCH],
                                start=(j == 0),
                                stop=(not with_bias and j == NPAIR - 1),
                                perf_mode=DR)
                    if with_bias:
                        for n in range(OUT // NCH):
                            nc.tensor.matmul(ps[:, n * NCH:(n + 1) * NCH],
                                             ones_f8[:],
                                             bias_f8[:, n * NCH:(n + 1) * NCH],
                                             start=False, stop=True)
                    return ps

                def emit_epilogue(it, ps, out_g):
                    t = it % G
                    dst = out_g[:, t * OUT:(t + 1) * OUT]
                    # engine balance: route-B tiles do pass1 on ACT
                    # (per-row scale fused into the PSUM->bf16 move via
                    # the per-partition scale AP) and pass2 (bf16 * ws)
                    # mostly on Pool; route-C tiles do psum*ws on DVE
                    # (f32) and the per-row scale as a bf16 tensor_scalar.
                    m16 = it % 16
                    if m16 < 13:
                        # ACT: psum -> bf16 with per-row |x| scale
                        pre = pre_pool.tile([P, OUT], BF16, tag="pre")
                        nc.scalar.activation(pre[:], ps[:], ACT.Copy,
                                             scale=is_raws[it][:])
                        # bf16 tensor_tensor: * ws (Pool for most,
                        # DVE 2x for the rest)
                        eng = nc.gpsimd if m16 < 9 else nc.vector
                        eng.tensor_tensor(dst, pre[:], ws_bcast_bf[:],
                                          op=ALU.mult)
                    else:
                        # DVE f32: psum * ws -> bf16
                        nc.vector.tensor_tensor(dst, ps[:], ws_bcast[:],
                                                op=ALU.mult)
                        # DVE bf16 in-place: * per-row |x| sum
                        nc.vector.tensor_scalar(dst, dst, is_raws[it][:],
                                                None, op0=ALU.mult)
                    # one deferred |x| row sum per tile so the DVE queue
                    # never holds a burst of reduces ahead of the
                    # psum-draining epilogue ops
                    emit_reduce(it + 2 * G)

                # ---- main loop; group 0 runs 2 tiles of matmuls before
                # the ws-prep PE chain, whose epilogues follow it ----
                for g in range(NG):
                    out_g = og_pool.tile([P, G * OUT], BF16, tag="og",
                                         name=f"og{g}")
                    if g == 0:
                        ps0 = emit_matmuls(0, 0)
                        ps1 = emit_matmuls(0, 1)
                        emit_ws_prep()
                        emit_epilogue(0, ps0, out_g)
                        emit_epilogue(1, ps1, out_g)
                        for t in range(2, G):
                            ps = emit_matmuls(0, t)
                            emit_epilogue(t, ps, out_g)
                    else:
                        for t in range(G):
                            ps = emit_matmuls(g, t)
                            emit_epilogue(g * G + t, ps, out_g)

                    # out DMA on the software-DGE queue (sync carries the
                    # XBAR stream, scalar the x-in stream)
                    eng = nc.sync if g == NG - 1 else nc.gpsimd
                    eng.dma_start(
                        o_ap[g * G * P:(g + 1) * G * P, :].rearrange(
                            "(u p) o -> p u o", p=P),
                        out_g[:].rearrange("p (u o) -> p u o", u=G))

                    if g + 3 < NG:
                        stage_in(g + 3)
                        sign_group(g + 3)

    nc.compile()
    return nc


def _get_nc(with_bias, with_gate):
    key = f"nc{int(with_bias)}{int(with_gate)}"
    if key not in _CACHE:
        _CACHE[key] = _build(with_bias, with_gate)
    return _CACHE[key]


def run(x, weight, gate, bias, trace=False):
    # gate >= 0 everywhere makes the gate mask exactly 1 ((sign(g)+1)/2 with
    # g==0 -> 0.5, and sign(w*0.5) == sign(w)); skip it entirely then.
    nc = _get_nc(bool(np.any(np.asarray(bias))),
                 not bool(np.all(np.asarray(gate) >= 0.0)))
    x = np.asarray(x, dtype=np.float32).astype(ml_dtypes.bfloat16)
    weight = np.asarray(weight, dtype=np.float32).astype(ml_dtypes.bfloat16)
    gate = np.asarray(gate, dtype=np.float32).astype(ml_dtypes.bfloat16)
    bias = np.ascontiguousarray(np.asarray(bias, dtype=np.float32)).reshape(1, OUT)
    in_maps = [
        {"x": x[i * BSH:(i + 1) * BSH], "weight": weight, "gate": gate, "bias": bias}
        for i in range(NCORES)
    ]
    res = run_bass_kernel_spmd(nc, in_maps, core_ids=list(range(NCORES)), trace=trace)
    out = np.concatenate([res.results[i]["out"] for i in range(NCORES)],
                         axis=0).astype(np.float32)
    return out, res


def kernel(x, weight, gate, bias):
    out, _ = run(x, weight, gate, bias, trace=False)
    return out


# revision 47
# speedup vs baseline: 1.0913x; 1.0913x over previous
"""Trainium2 Bass kernel for nn_BitwiseLinear (8 NeuronCores, SPMD).

Reference semantics (B=32768, IN=OUT=1024):
    out = in_scale * weight_scale * (sign(x) @ sign(weight * gate_mask).T + bias)
    gate_mask = (sign(gate)+1)/2; in_scale = mean|x| per row; weight_scale = mean|w| per out.

Identities used:
    sign(weight * gate_mask) == sign(weight) * (gate >= 0)  (gate==0 -> mask 0.5 -> sign(w))
    out = sum|x|_row * (signmm + bias) * ws_eff,  ws_eff = sum|w|_row * 2^-20

Sharding: data-parallel on batch across the 8 cores (x/out sharded 4096 rows
per core), weight replicated, no collectives.

Design (vs the f32 PE-transpose baseline):
  * x and weight are cast to bf16 on the host (2e-2 tolerance; bf16 costs
    ~0.25%): halves the x/w HBM reads; output written bf16, upcast on host.
  * All x/w transposes moved off the PE onto the DMA XBAR transpose
    (2-byte dtype): weight is transposed [o,i]->[i,o] straight from DRAM
    in two k-half descriptors on the scalar hwdge queue; x groups go
    DRAM->SBUF plain on the scalar queue (the [b,i] layout feeds the
    per-row |x| sums on DVE) and SBUF->SBUF through the XBAR on the sync
    queue. (DRAM-sourced XBAR of the x shards corrupted data on
    even-numbered cores, so x stays SBUF-sourced.)
  * PE runs only the 512 fp8 DoubleRow matmuls (sign(x)^T stationary,
    [P, 2*NCH] psum tiles spanning two banks, 4 in flight).
  * Epilogue split for engine balance: most tiles fold the per-row |x|
    sum into the PSUM->bf16 move on ACT (per-partition scale AP) then
    multiply by ws in bf16 on Pool/DVE; the rest do psum*ws on DVE (f32)
    plus an in-place bf16 tensor_scalar. Row-sum reduces are emitted one
    per tile, two groups ahead, so the DVE queue never delays the
    psum-draining ops.
  Runtime specialization: bias matmuls dropped when bias is all-zero and the
  gate path dropped when gate >= 0 everywhere (checked against the actual
  inputs; other variants compile lazily and remain correct).
"""

import numpy as np
import ml_dtypes

import concourse.bacc as bacc
import concourse.mybir as mybir
import concourse.tile as tile
from concourse import masks
from concourse.bass_utils import run_bass_kernel_spmd

B, IN, OUT = 32768, 1024, 1024
NCORES = 8
BSH = B // NCORES            # 4096 rows per core
P = 128                      # partitions
NT = BSH // P                # 32 x-tiles per core
G = 4                        # x-tiles per DMA/XBAR group
NG = NT // G                 # 8 groups
KC = IN // P                 # 8 contraction chunks of 128
NPAIR = KC // 2              # 4 DoubleRow K-pairs (256 each)
NCH = 512                    # matmul moving free-dim (one PSUM bank of f32)
F32 = mybir.dt.float32
BF16 = mybir.dt.bfloat16
FP8 = mybir.dt.float8e4
WS_SCALE = float(2.0 ** -20)  # 1/(1024*1024): folds both mean divisors

_CACHE: dict = {}


def _build(with_bias=True, with_gate=True):
    nc = bacc.Bacc("TRN2", target_bir_lowering=False, debug=False,
                   num_devices=NCORES)

    x_ext = nc.declare_dram_parameter("x", [BSH, IN], BF16, isOutput=False)
    w_ext = nc.declare_dram_parameter("weight", [OUT, IN], BF16, isOutput=False)
    g_ext = nc.declare_dram_parameter("gate", [OUT, IN], BF16, isOutput=False)
    b_ext = nc.declare_dram_parameter("bias", [1, OUT], F32, isOutput=False)
    o_ext = nc.declare_dram_parameter("out", [BSH, OUT], BF16, isOutput=True)

    x_ap = x_ext.ap()
    w_ap = w_ext.ap()
    g_ap = g_ext.ap()
    b_ap = b_ext.ap()
    o_ap = o_ext.ap()

    ACT = mybir.ActivationFunctionType
    ALU = mybir.AluOpType
    AX = mybir.AxisListType
    DR = mybir.MatmulPerfMode.DoubleRow

    with tile.TileContext(nc) as tc:
        with tc.tile_pool(name="const", bufs=1) as cp:
            ident_f32 = cp.tile([P, P], F32)
            ones_f32 = cp.tile([1, P], F32)
            zbias = cp.tile([P, 1], F32)

            # persistent prepped weights: wT chunks c at [:, c*OUT:(c+1)*OUT]
            # (chunk c holds w[o, c*128+p] for all o)
            wT_bf = cp.tile([P, KC * OUT], BF16, name="wT_bf")
            wtq = cp.tile([P, KC * OUT], FP8, name="wtq")
            ws_bcast = cp.tile([P, OUT], F32)     # ws * 2^-20 broadcast
            ws_bcast_bf = cp.tile([P, OUT], BF16)
            if with_bias:
                ones_f8 = cp.tile([1, P], FP8)
                bias_f8 = cp.tile([1, OUT], FP8)

            wtq_v = wtq[:].rearrange("p (c o) -> p c o", c=KC)

            with tc.tile_pool(name="wk", bufs=1) as wk, \
                 tc.tile_pool(name="xin", bufs=4) as xin_pool, \
                 tc.tile_pool(name="xtr", bufs=4) as xtr_pool, \
                 tc.tile_pool(name="xbt", bufs=4) as xbt_pool, \
                 tc.tile_pool(name="pre", bufs=4) as pre_pool, \
                 tc.tile_pool(name="og", bufs=3) as og_pool, \
                 tc.tile_pool(name="sc", bufs=16) as sc_pool, \
                 tc.tile_pool(name="pso", bufs=4, space="PSUM") as pso_pool:

                xgs = [None] * NG
                xTs = [None] * NG
                sxTs = [None] * NG
                is_raws = [None] * NT

                def stage_in(g):
                    """x group g: plain DMA in (scalar hwdge), then SBUF->
                    SBUF XBAR transpose on the sync queue (DRAM-sourced
                    XBAR transposes corrupt data on even-numbered cores).
                    Resulting layout: [p, t*KC+c, b]."""
                    xg = xin_pool.tile([P, G * IN], BF16, tag="xg",
                                       name=f"xg{g}")
                    nc.scalar.dma_start(
                        xg[:].rearrange("p (t i) -> p t i", t=G),
                        x_ap[g * G * P:(g + 1) * G * P, :].rearrange(
                            "(t p) i -> p t i", p=P))
                    xT = xtr_pool.tile([P, KC * G * P], BF16, tag="xT",
                                       name=f"xT{g}")
                    nc.sync.dma_start(
                        out=xT[:].rearrange("p (c m) -> p c m", c=G * KC),
                        in_=xg[:],
                        transpose=True)
                    xgs[g] = xg
                    xTs[g] = xT

                def emit_reduce(it):
                    """per-row |x| sum for tile it; deferred into the main
                    loop so the DVE queue isn't clogged ahead of the
                    psum-draining epilogue ops."""
                    if it >= NT or is_raws[it] is not None:
                        return
                    g, t = divmod(it, G)
                    is_raw = sc_pool.tile([P, 1], F32, tag="is_raw",
                                          name=f"is_raw{it}")
                    nc.vector.tensor_reduce(
                        is_raw[:], xgs[g][:, t * IN:(t + 1) * IN],
                        axis=AX.X, op=ALU.add,
                        apply_absolute_value=True)
                    is_raws[it] = is_raw

                def sign_group(g):
                    sxT = xbt_pool.tile([P, KC * G * P], FP8, tag="sxT",
                                        name=f"sxT{g}")
                    nc.scalar.activation(sxT[:], xTs[g][:], ACT.Sign,
                                         bias=zbias[:])
                    sxTs[g] = sxT

                masks.make_identity(nc, ident_f32[:])
                nc.gpsimd.memset(ones_f32[:], 1.0)
                nc.gpsimd.memset(zbias[:], 0.0)
                if with_bias:
                    nc.gpsimd.memset(ones_f8[:], 1.0)

                # ---- prologue: x group 0 first on the scalar queue, then
                # the [o, i] -> [i, o] weight XBAR in two k-chunk halves so
                # Sign(w, half 0) lands as early as possible ----
                stage_in(0)
                wT_v = wT_bf[:].rearrange("p (c o) -> p c o", c=KC)
                for h in range(2):
                    nc.scalar.dma_start(
                        out=wT_v[:, h * 4:(h + 1) * 4, :],
                        in_=w_ap[:, h * IN // 2:(h + 1) * IN // 2],
                        transpose=True)
                stage_in(1)
                stage_in(2)
                # plain copy of w only to compute the per-o |w| row sums
                wt4 = [wk.tile([P, 4 * IN], BF16, tag=f"wt4_{q}",
                               name=f"wt4_{q}") for q in range(2)]
                for q in range(2):
                    nc.gpsimd.dma_start(
                        wt4[q][:].rearrange("p (t i) -> p t i", t=4),
                        w_ap[q * 4 * P:(q + 1) * 4 * P, :].rearrange(
                            "(t p) i -> p t i", p=P))
                ws_cols = wk.tile([P, KC], F32)   # per-o |w| row sums
                ws_row = wk.tile([1, OUT], F32)
                if with_bias:
                    bias_sb = wk.tile([1, OUT], F32)
                    nc.sync.dma_start(bias_sb[:], b_ap[:, :])
                    nc.vector.tensor_copy(bias_f8[:], bias_sb[:])
                if with_gate:
                    gT_bf = wk.tile([P, KC * OUT], BF16, name="gT_bf")
                    nc.scalar.dma_start(
                        out=gT_bf[:].rearrange("p (c o) -> p c o", c=KC),
                        in_=g_ap[:, :],
                        transpose=True)

                # weight sign binarization on ACT, k-chunk halves in j-order,
                # interleaved with the first x Signs on the in-order ACT
                # queue so matmuls(g0, j=0..1) start as early as possible.
                if with_gate:
                    msk = wk.tile([P, KC * OUT], BF16, name="gmask")
                    nc.vector.tensor_scalar(msk[:], gT_bf[:], 0.0, None,
                                            op0=ALU.is_ge)
                    wTm = wk.tile([P, KC * OUT], BF16, name="wTm")
                    nc.vector.tensor_tensor(wTm[:], wT_bf[:], msk[:],
                                            op=ALU.mult)
                    w_src = wTm
                else:
                    w_src = wT_bf
                half = KC * OUT // 2
                nc.scalar.activation(wtq[:, :half], w_src[:, :half], ACT.Sign,
                                     bias=zbias[:])
                sign_group(0)
                nc.scalar.activation(wtq[:, half:], w_src[:, half:], ACT.Sign,
                                     bias=zbias[:])
                sign_group(1)
                sign_group(2)
                # per-o |w| sums first on the DVE queue: they gate the ws
                # PE ops; the x row sums only feed epilogues
                for t in range(KC):
                    nc.vector.tensor_reduce(
                        ws_cols[:, t:t + 1],
                        wt4[t // 4][:, (t % 4) * IN:((t % 4) + 1) * IN],
                        axis=AX.X, op=ALU.add, apply_absolute_value=True)
                for it in range(2 * G):
                    emit_reduce(it)

                def emit_ws_prep():
                    """ws_row[0, o] = sum_i |w[o, i]| * 2^-20 via tiny PE
                    transposes + K=1 broadcast matmuls. Emitted mid-loop
                    (after 2 x-tiles of matmuls, before their epilogues) so
                    this chain never heads the in-order PE queue and stalls
                    it on the w-reduce dependencies. Exactly 2 psum ring
                    slots so tiles 0/1's psum isn't recycled early."""
                    ps_wt = pso_pool.tile([P, 2 * NCH], F32, tag="ps_o",
                                          name="ps_row")
                    for t in range(KC):
                        nc.tensor.transpose(ps_wt[0:1, t * P:(t + 1) * P],
                                            ws_cols[:, t:t + 1],
                                            ident_f32[:])
                    nc.scalar.activation(ws_row[:], ps_wt[0:1, :],
                                         ACT.Copy, scale=WS_SCALE)
                    ps_bc = pso_pool.tile([P, 2 * NCH], F32, tag="ps_o",
                                          name="ps_bc")
                    for n in range(OUT // NCH):
                        nc.tensor.matmul(ps_bc[:, n * NCH:(n + 1) * NCH],
                                         ones_f32[:],
                                         ws_row[:, n * NCH:(n + 1) * NCH])
                    nc.vector.tensor_copy(ws_bcast[:], ps_bc[:])
                    nc.vector.tensor_copy(ws_bcast_bf[:], ws_bcast[:])

                def emit_matmuls(g, t):
                    # one [P, 1024] f32 psum tile = two adjacent banks;
                    # each matmul writes within a single bank, epilogue
                    # ops span both in one instruction
                    sxv = sxTs[g][:].rearrange("p (c m) -> p c m", c=G * KC)
                    ps = pso_pool.tile([P, 2 * NCH], F32, tag="ps_o")
                    for j in range(NPAIR):
                        xp = sxv[:, t * KC + 2 * j:t * KC + 2 * j + 2, :]
                        wq = wtq_v[:, 2 * j:2 * j + 2, :]
                        for n in range(OUT // NCH):
                            nc.tensor.matmul(
                                ps[:, n * NCH:(n + 1) * NCH],
                                xp,
                                wq[:, :, n * NCH:(n + 1) * NCH],
                                start=(j == 0),
                                stop=(not with_bias and j == NPAIR - 1),
                                perf_mode=DR)
                    if with_bias:
                        for n in range(OUT // NCH):
                            nc.tensor.matmul(ps[:, n * NCH:(n + 1) * NCH],
                                             ones_f8[:],
                                             bias_f8[:, n * NCH:(n + 1) * NCH],
                                             start=False, stop=True)
                    return ps

                def emit_epilogue(it, ps, out_g):
                    t = it % G
                    dst = out_g[:, t * OUT:(t + 1) * OUT]
                    # engine balance: route-B tiles do pass1 on ACT
                    # (per-row scale fused into the PSUM->bf16 move via
                    # the per-partition scale AP) and pass2 (bf16 * ws)
                    # mostly on Pool; route-C tiles do psum*ws on DVE
                    # (f32) and the per-row scale as a bf16 tensor_scalar.
                    m16 = it % 16
                    if m16 < 13:
                        # ACT: psum -> bf16 with per-row |x| scale
                        pre = pre_pool.tile([P, OUT], BF16, tag="pre")
                        nc.scalar.activation(pre[:], ps[:], ACT.Copy,
                                             scale=is_raws[it][:])
                        # bf16 tensor_tensor: * ws (Pool for most,
                        # DVE 2x for the rest)
                        eng = nc.gpsimd if m16 < 9 else nc.vector
                        eng.tensor_tensor(dst, pre[:], ws_bcast_bf[:],
                                          op=ALU.mult)
                    else:
                        # DVE f32: psum * ws -> bf16
                        nc.vector.tensor_tensor(dst, ps[:], ws_bcast[:],
                                                op=ALU.mult)
                        # DVE bf16 in-place: * per-row |x| sum
                        nc.vector.tensor_scalar(dst, dst, is_raws[it][:],
                                                None, op0=ALU.mult)
                    # one deferred |x| row sum per tile so the DVE queue
                    # never holds a burst of reduces ahead of the
                    # psum-draining epilogue ops
                    emit_reduce(it + 2 * G)

                # ---- main loop. ws-prep leads the PE queue; its w-reduce
                # dependencies were put at the head of the DVE queue so it
                # clears in ~2us instead of stalling the matmul stream. ----
                emit_ws_prep()
                for g in range(NG):
                    out_g = og_pool.tile([P, G * OUT], BF16, tag="og",
                                         name=f"og{g}")
                    for t in range(G):
                        ps = emit_matmuls(g, t)
                        emit_epilogue(g * G + t, ps, out_g)

                    # out DMA on the software-DGE queue (sync carries the
                    # XBAR stream, scalar the x-in stream)
                    eng = nc.sync if g == NG - 1 else nc.gpsimd
                    eng.dma_start(
                        o_ap[g * G * P:(g + 1) * G * P, :].rearrange(
                            "(u p) o -> p u o", p=P),
                        out_g[:].rearrange("p (u o) -> p u o", u=G))

                    if g + 3 < NG:
                        stage_in(g + 3)
                        sign_group(g + 3)

    nc.compile()
    return nc


def _get_nc(with_bias, with_gate):
    key = f"nc{int(with_bias)}{int(with_gate)}"
    if key not in _CACHE:
        _CACHE[key] = _build(with_bias, with_gate)
    return _CACHE[key]


def run(x, weight, gate, bias, trace=False):
    # gate >= 0 everywhere makes the gate mask exactly 1 ((sign(g)+1)/2 with
    # g==0 -> 0.5, and sign(w*0.5) == sign(w)); skip it entirely then.
    nc = _get_nc(bool(np.any(np.asarray(bias))),
                 not bool(np.all(np.asarray(gate) >= 0.0)))
    x = np.asarray(x, dtype=np.float32).astype(ml_dtypes.bfloat16)
    weight = np.asarray(weight, dtype=np.float32).astype(ml_dtypes.bfloat16)
    gate = np.asarray(gate, dtype=np.float32).astype(ml_dtypes.bfloat16)
    bias = np.ascontiguousarray(np.asarray(bias, dtype=np.float32)).reshape(1, OUT)
    in_maps = [
        {"x": x[i * BSH:(i + 1) * BSH], "weight": weight, "gate": gate, "bias": bias}
        for i in range(NCORES)
    ]
    res = run_bass_kernel_spmd(nc, in_maps, core_ids=list(range(NCORES)), trace=trace)
    out = np.concatenate([res.results[i]["out"] for i in range(NCORES)],
                         axis=0).astype(np.float32)
    return out, res


def kernel(x, weight, gate, bias):
    out, _ = run(x, weight, gate, bias, trace=False)
    return out


# revision 48
# speedup vs baseline: 1.1400x; 1.0446x over previous
"""Trainium2 Bass kernel for nn_BitwiseLinear (8 NeuronCores, SPMD).

Reference semantics (B=32768, IN=OUT=1024):
    out = in_scale * weight_scale * (sign(x) @ sign(weight * gate_mask).T + bias)
    gate_mask = (sign(gate)+1)/2; in_scale = mean|x| per row; weight_scale = mean|w| per out.

Identities used:
    sign(weight * gate_mask) == sign(weight) * (gate >= 0)  (gate==0 -> mask 0.5 -> sign(w))
    out = sum|x|_row * (signmm + bias) * ws_eff,  ws_eff = sum|w|_row * 2^-20

Sharding: data-parallel on batch across the 8 cores (x/out sharded 4096 rows
per core), weight replicated, no collectives.

Design (vs the f32 PE-transpose baseline):
  * x and weight are cast to bf16 on the host (2e-2 tolerance; bf16 costs
    ~0.25%): halves the x/w HBM reads; output written bf16, upcast on host.
  * All x/w transposes moved off the PE onto the DMA XBAR transpose
    (2-byte dtype): weight is transposed [o,i]->[i,o] straight from DRAM
    in two k-half descriptors on the scalar hwdge queue; x groups go
    DRAM->SBUF plain on the scalar queue (the [b,i] layout feeds the
    per-row |x| sums on DVE) and SBUF->SBUF through the XBAR on the sync
    queue. (DRAM-sourced XBAR of the x shards corrupted data on
    even-numbered cores, so x stays SBUF-sourced.)
  * PE runs only the 512 fp8 DoubleRow matmuls (sign(x)^T stationary,
    [P, 2*NCH] psum tiles spanning two banks, 4 in flight).
  * Epilogue split for engine balance: most tiles fold the per-row |x|
    sum into the PSUM->bf16 move on ACT (per-partition scale AP) then
    multiply by ws in bf16 on Pool/DVE; the rest do psum*ws on DVE (f32)
    plus an in-place bf16 tensor_scalar. Row-sum reduces are emitted one
    per tile, two groups ahead, so the DVE queue never delays the
    psum-draining ops.
  Runtime specialization: bias matmuls dropped when bias is all-zero and the
  gate path dropped when gate >= 0 everywhere (checked against the actual
  inputs; other variants compile lazily and remain correct).
"""

import numpy as np
import ml_dtypes

import concourse.bacc as bacc
import concourse.mybir as mybir
import concourse.tile as tile
from concourse import masks
from concourse.bass_utils import run_bass_kernel_spmd

B, IN, OUT = 32768, 1024, 1024
NCORES = 8
BSH = B // NCORES            # 4096 rows per core
P = 128                      # partitions
NT = BSH // P                # 32 x-tiles per core
G = 4                        # x-tiles per DMA/XBAR group
NG = NT // G                 # 8 groups
KC = IN // P                 # 8 contraction chunks of 128
NPAIR = KC // 2              # 4 DoubleRow K-pairs (256 each)
NCH = 512                    # matmul moving free-dim (one PSUM bank of f32)
F32 = mybir.dt.float32
BF16 = mybir.dt.bfloat16
FP8 = mybir.dt.float8e4
WS_SCALE = float(2.0 ** -20)  # 1/(1024*1024): folds both mean divisors

_CACHE: dict = {}


def _build(with_bias=True, with_gate=True):
    nc = bacc.Bacc("TRN2", target_bir_lowering=False, debug=False,
                   num_devices=NCORES)

    x_ext = nc.declare_dram_parameter("x", [BSH, IN], BF16, isOutput=False)
    w_ext = nc.declare_dram_parameter("weight", [OUT, IN], BF16, isOutput=False)
    g_ext = nc.declare_dram_parameter("gate", [OUT, IN], BF16, isOutput=False)
    b_ext = nc.declare_dram_parameter("bias", [1, OUT], F32, isOutput=False)
    o_ext = nc.declare_dram_parameter("out", [BSH, OUT], BF16, isOutput=True)

    x_ap = x_ext.ap()
    w_ap = w_ext.ap()
    g_ap = g_ext.ap()
    b_ap = b_ext.ap()
    o_ap = o_ext.ap()

    ACT = mybir.ActivationFunctionType
    ALU = mybir.AluOpType
    AX = mybir.AxisListType
    DR = mybir.MatmulPerfMode.DoubleRow

    with tile.TileContext(nc) as tc:
        with tc.tile_pool(name="const", bufs=1) as cp:
            ident_f32 = cp.tile([P, P], F32)
            ones_f32 = cp.tile([1, P], F32)
            zbias = cp.tile([P, 1], F32)

            # persistent prepped weights: wT chunks c at [:, c*OUT:(c+1)*OUT]
            # (chunk c holds w[o, c*128+p] for all o)
            wT_bf = cp.tile([P, KC * OUT], BF16, name="wT_bf")
            wtq = cp.tile([P, KC * OUT], FP8, name="wtq")
            ws_bcast = cp.tile([P, OUT], F32)     # ws * 2^-20 broadcast
            ws_bcast_bf = cp.tile([P, OUT], BF16)
            if with_bias:
                ones_f8 = cp.tile([1, P], FP8)
                bias_f8 = cp.tile([1, OUT], FP8)

            wtq_v = wtq[:].rearrange("p (c o) -> p c o", c=KC)

            with tc.tile_pool(name="wk", bufs=1) as wk, \
                 tc.tile_pool(name="xin", bufs=4) as xin_pool, \
                 tc.tile_pool(name="xtr", bufs=4) as xtr_pool, \
                 tc.tile_pool(name="xbt", bufs=4) as xbt_pool, \
                 tc.tile_pool(name="pre", bufs=4) as pre_pool, \
                 tc.tile_pool(name="og", bufs=3) as og_pool, \
                 tc.tile_pool(name="sc", bufs=16) as sc_pool, \
                 tc.tile_pool(name="pso", bufs=4, space="PSUM") as pso_pool:

                xgs = [None] * NG
                xTs = [None] * NG
                sxTs = [None] * NG
                is_raws = [None] * NT

                def stage_in(g):
                    """x group g: plain DMA in (scalar hwdge), then SBUF->
                    SBUF XBAR transpose on the sync queue (DRAM-sourced
                    XBAR transposes corrupt data on even-numbered cores).
                    Resulting layout: [p, t*KC+c, b]."""
                    xg = xin_pool.tile([P, G * IN], BF16, tag="xg",
                                       name=f"xg{g}")
                    nc.scalar.dma_start(
                        xg[:].rearrange("p (t i) -> p t i", t=G),
                        x_ap[g * G * P:(g + 1) * G * P, :].rearrange(
                            "(t p) i -> p t i", p=P))
                    xT = xtr_pool.tile([P, KC * G * P], BF16, tag="xT",
                                       name=f"xT{g}")
                    nc.sync.dma_start(
                        out=xT[:].rearrange("p (c m) -> p c m", c=G * KC),
                        in_=xg[:],
                        transpose=True)
                    xgs[g] = xg
                    xTs[g] = xT

                def emit_reduce(it):
                    """per-row |x| sum for tile it; deferred into the main
                    loop so the DVE queue isn't clogged ahead of the
                    psum-draining epilogue ops."""
                    if it >= NT or is_raws[it] is not None:
                        return
                    g, t = divmod(it, G)
                    is_raw = sc_pool.tile([P, 1], F32, tag="is_raw",
                                          name=f"is_raw{it}")
                    nc.vector.tensor_reduce(
                        is_raw[:], xgs[g][:, t * IN:(t + 1) * IN],
                        axis=AX.X, op=ALU.add,
                        apply_absolute_value=True)
                    is_raws[it] = is_raw

                def sign_group(g):
                    sxT = xbt_pool.tile([P, KC * G * P], FP8, tag="sxT",
                                        name=f"sxT{g}")
                    nc.scalar.activation(sxT[:], xTs[g][:], ACT.Sign,
                                         bias=zbias[:])
                    sxTs[g] = sxT

                masks.make_identity(nc, ident_f32[:])
                nc.gpsimd.memset(ones_f32[:], 1.0)
                nc.gpsimd.memset(zbias[:], 0.0)
                if with_bias:
                    nc.gpsimd.memset(ones_f8[:], 1.0)

                # ---- prologue: x group 0 first on the scalar queue, then
                # the [o, i] -> [i, o] weight XBAR in two k-chunk halves so
                # Sign(w, half 0) lands as early as possible ----
                stage_in(0)
                wT_v = wT_bf[:].rearrange("p (c o) -> p c o", c=KC)
                for h in range(2):
                    nc.scalar.dma_start(
                        out=wT_v[:, h * 4:(h + 1) * 4, :],
                        in_=w_ap[:, h * IN // 2:(h + 1) * IN // 2],
                        transpose=True)
                stage_in(1)
                stage_in(2)
                # plain copy of w only to compute the per-o |w| row sums
                wt4 = [wk.tile([P, 4 * IN], BF16, tag=f"wt4_{q}",
                               name=f"wt4_{q}") for q in range(2)]
                for q in range(2):
                    nc.gpsimd.dma_start(
                        wt4[q][:].rearrange("p (t i) -> p t i", t=4),
                        w_ap[q * 4 * P:(q + 1) * 4 * P, :].rearrange(
                            "(t p) i -> p t i", p=P))
                ws_cols = wk.tile([P, KC], F32)   # per-o |w| row sums
                ws_row = wk.tile([1, OUT], F32)
                if with_bias:
                    bias_sb = wk.tile([1, OUT], F32)
                    nc.sync.dma_start(bias_sb[:], b_ap[:, :])
                    nc.vector.tensor_copy(bias_f8[:], bias_sb[:])
                if with_gate:
                    gT_bf = wk.tile([P, KC * OUT], BF16, name="gT_bf")
                    nc.scalar.dma_start(
                        out=gT_bf[:].rearrange("p (c o) -> p c o", c=KC),
                        in_=g_ap[:, :],
                        transpose=True)

                # weight sign binarization on ACT, k-chunk halves in j-order,
                # interleaved with the first x Signs on the in-order ACT
                # queue so matmuls(g0, j=0..1) start as early as possible.
                if with_gate:
                    msk = wk.tile([P, KC * OUT], BF16, name="gmask")
                    nc.vector.tensor_scalar(msk[:], gT_bf[:], 0.0, None,
                                            op0=ALU.is_ge)
                    wTm = wk.tile([P, KC * OUT], BF16, name="wTm")
                    nc.vector.tensor_tensor(wTm[:], wT_bf[:], msk[:],
                                            op=ALU.mult)
                    w_src = wTm
                else:
                    w_src = wT_bf
                half = KC * OUT // 2
                nc.scalar.activation(wtq[:, :half], w_src[:, :half], ACT.Sign,
                                     bias=zbias[:])
                sign_group(0)
                nc.scalar.activation(wtq[:, half:], w_src[:, half:], ACT.Sign,
                                     bias=zbias[:])
                sign_group(1)
                sign_group(2)
                for it in range(2 * G):
                    emit_reduce(it)
                # per-o |w| sums (after the x row sums: early is_raw
                # matters more — late ws_cols only idles the PE head,
                # which measured cheaper than stalled epilogues)
                for t in range(KC):
                    nc.vector.tensor_reduce(
                        ws_cols[:, t:t + 1],
                        wt4[t // 4][:, (t % 4) * IN:((t % 4) + 1) * IN],
                        axis=AX.X, op=ALU.add, apply_absolute_value=True)

                def emit_ws_prep():
                    """ws_row[0, o] = sum_i |w[o, i]| * 2^-20 via tiny PE
                    transposes + K=1 broadcast matmuls. Emitted mid-loop
                    (after 2 x-tiles of matmuls, before their epilogues) so
                    this chain never heads the in-order PE queue and stalls
                    it on the w-reduce dependencies. Exactly 2 psum ring
                    slots so tiles 0/1's psum isn't recycled early."""
                    ps_wt = pso_pool.tile([P, 2 * NCH], F32, tag="ps_o",
                                          name="ps_row")
                    for t in range(KC):
                        nc.tensor.transpose(ps_wt[0:1, t * P:(t + 1) * P],
                                            ws_cols[:, t:t + 1],
                                            ident_f32[:])
                    nc.scalar.activation(ws_row[:], ps_wt[0:1, :],
                                         ACT.Copy, scale=WS_SCALE)
                    ps_bc = pso_pool.tile([P, 2 * NCH], F32, tag="ps_o",
                                          name="ps_bc")
                    for n in range(OUT // NCH):
                        nc.tensor.matmul(ps_bc[:, n * NCH:(n + 1) * NCH],
                                         ones_f32[:],
                                         ws_row[:, n * NCH:(n + 1) * NCH])
                    nc.vector.tensor_copy(ws_bcast[:], ps_bc[:])
                    nc.vector.tensor_copy(ws_bcast_bf[:], ws_bcast[:])

                def emit_matmuls(g, t):
                    # one [P, 1024] f32 psum tile = two adjacent banks;
                    # each matmul writes within a single bank, epilogue
                    # ops span both in one instruction
                    sxv = sxTs[g][:].rearrange("p (c m) -> p c m", c=G * KC)
                    ps = pso_pool.tile([P, 2 * NCH], F32, tag="ps_o")
                    for j in range(NPAIR):
                        xp = sxv[:, t * KC + 2 * j:t * KC + 2 * j + 2, :]
                        wq = wtq_v[:, 2 * j:2 * j + 2, :]
                        for n in range(OUT // NCH):
                            nc.tensor.matmul(
                                ps[:, n * NCH:(n + 1) * NCH],
                                xp,
                                wq[:, :, n * NCH:(n + 1) * NCH],
                                start=(j == 0),
                                stop=(not with_bias and j == NPAIR - 1),
                                perf_mode=DR)
                    if with_bias:
                        for n in range(OUT // NCH):
                            nc.tensor.matmul(ps[:, n * NCH:(n + 1) * NCH],
                                             ones_f8[:],
                                             bias_f8[:, n * NCH:(n + 1) * NCH],
                                             start=False, stop=True)
                    return ps

                def emit_epilogue(it, ps, out_g):
                    t = it % G
                    dst = out_g[:, t * OUT:(t + 1) * OUT]
                    # engine balance: route-B tiles do pass1 on ACT
                    # (per-row scale fused into the PSUM->bf16 move via
                    # the per-partition scale AP) and pass2 (bf16 * ws)
                    # mostly on Pool; route-C tiles do psum*ws on DVE
                    # (f32) and the per-row scale as a bf16 tensor_scalar.
                    m16 = it % 16
                    if m16 < 13:
                        # ACT: psum -> bf16 with per-row |x| scale
                        pre = pre_pool.tile([P, OUT], BF16, tag="pre")
                        nc.scalar.activation(pre[:], ps[:], ACT.Copy,
                                             scale=is_raws[it][:])
                        # bf16 tensor_tensor: * ws (Pool for most,
                        # DVE 2x for the rest)
                        eng = nc.gpsimd if m16 < 9 else nc.vector
                        eng.tensor_tensor(dst, pre[:], ws_bcast_bf[:],
                                          op=ALU.mult)
                    else:
                        # DVE f32: psum * ws -> bf16
                        nc.vector.tensor_tensor(dst, ps[:], ws_bcast[:],
                                                op=ALU.mult)
                        # DVE bf16 in-place: * per-row |x| sum
                        nc.vector.tensor_scalar(dst, dst, is_raws[it][:],
                                                None, op0=ALU.mult)
                    # one deferred |x| row sum per tile so the DVE queue
                    # never holds a burst of reduces ahead of the
                    # psum-draining epilogue ops
                    emit_reduce(it + 2 * G)

                # ---- main loop. ws-prep leads the PE queue; its w-reduce
                # dependencies were put at the head of the DVE queue so it
                # clears in ~2us instead of stalling the matmul stream. ----
                emit_ws_prep()
                for g in range(NG):
                    out_g = og_pool.tile([P, G * OUT], BF16, tag="og",
                                         name=f"og{g}")
                    for t in range(G):
                        ps = emit_matmuls(g, t)
                        emit_epilogue(g * G + t, ps, out_g)

                    # out DMA on the software-DGE queue (sync carries the
                    # XBAR stream, scalar the x-in stream)
                    eng = nc.sync if g == NG - 1 else nc.gpsimd
                    eng.dma_start(
                        o_ap[g * G * P:(g + 1) * G * P, :].rearrange(
                            "(u p) o -> p u o", p=P),
                        out_g[:].rearrange("p (u o) -> p u o", u=G))

                    if g + 3 < NG:
                        stage_in(g + 3)
                        sign_group(g + 3)

    nc.compile()
    return nc


def _get_nc(with_bias, with_gate):
    key = f"nc{int(with_bias)}{int(with_gate)}"
    if key not in _CACHE:
        _CACHE[key] = _build(with_bias, with_gate)
    return _CACHE[key]


def run(x, weight, gate, bias, trace=False):
    # gate >= 0 everywhere makes the gate mask exactly 1 ((sign(g)+1)/2 with
    # g==0 -> 0.5, and sign(w*0.5) == sign(w)); skip it entirely then.
    nc = _get_nc(bool(np.any(np.asarray(bias))),
                 not bool(np.all(np.asarray(gate) >= 0.0)))
    x = np.asarray(x, dtype=np.float32).astype(ml_dtypes.bfloat16)
    weight = np.asarray(weight, dtype=np.float32).astype(ml_dtypes.bfloat16)
    gate = np.asarray(gate, dtype=np.float32).astype(ml_dtypes.bfloat16)
    bias = np.ascontiguousarray(np.asarray(bias, dtype=np.float32)).reshape(1, OUT)
    in_maps = [
        {"x": x[i * BSH:(i + 1) * BSH], "weight": weight, "gate": gate, "bias": bias}
        for i in range(NCORES)
    ]
    res = run_bass_kernel_spmd(nc, in_maps, core_ids=list(range(NCORES)), trace=trace)
    out = np.concatenate([res.results[i]["out"] for i in range(NCORES)],
                         axis=0).astype(np.float32)
    return out, res


def kernel(x, weight, gate, bias):
    out, _ = run(x, weight, gate, bias, trace=False)
    return out
